# revision 1
# baseline (speedup 1.0000x reference)
"""Multi-head causal self-attention on 8 Trainium2 NeuronCores.

Problem: B=4, S=2048, D=1024, H=16 heads (dk=64), fp32, causal softmax.

Sharding: hybrid batch x head-group. Core c handles batch b = c//2 and head
group g = c%2 (8 heads = 512 dims). Each core computes QKV projections for
its head group, causal flash-style attention in scores-transposed layout,
and a partial output projection over its 512 context dims. The host sums
the two partials per batch.

Device-side layout choices (per core):
  - All matmuls run in float32r (TF32-like, ~1e-4 rel err, full PE speed).
  - Q^T, K^T stored [dk-major]: tile [128, 4, 2048]; partition block of 128
    = one head PAIR (64 rows head 2j, 64 rows head 2j+1) so score matmuls
    (contraction dk=64) row-pack two heads concurrently in the PE array.
  - Scores computed transposed: S^T[k, q] tiles [128 k, 512 q], exp on the
    scalar engine, causal handling by narrowing matmuls to q >= k-chunk
    start plus one triangular 128x128 mask multiply per diagonal block.
  - V stored [k-part, dv] with a ones-column appended (65 wide): the P@V
    matmul (lhsT=V_aug, rhs=exp(S^T)) then yields both the context and the
    softmax denominator (row 64) in one pass, accumulated over k chunks in
    PSUM. Normalization is deferred to after P@V.
  - Denominator reciprocal is broadcast across the 128 partitions of a head
    pair with a tiny [2,128] ones-pattern matmul, then applied with one
    vector multiply per context tile.
"""

import numpy as np
from contextlib import ExitStack

import concourse.bass as bass
import concourse.tile as tile
from concourse import bacc, mybir
from concourse.bass_utils import run_bass_kernel_spmd

B, S, D = 4, 2048, 1024
H = 16
DK = 64
G = 2              # head groups (cores per batch)
HD = D // G        # per-core head dims = 512 (8 heads)
NH = HD // DK      # heads per core = 8
P = 128
NPAIR = NH // 2    # head pairs per core = 4
QC = 512           # q free-dim chunk
NQC = S // QC      # 4
NKC = S // P       # 16 k chunks
KO = D // P        # 8 contraction chunks for projections

F32 = mybir.dt.float32
F32R = mybir.dt.float32r
EXP = mybir.ActivationFunctionType.Exp

_CACHE: dict = {}


def _emit(ctx: ExitStack, tc, xt, wq, wk, wv, wo, tri, m2, ones, out):
    nc = tc.nc

    # ---- persistent SBUF tensors -------------------------------------
    persist = ctx.enter_context(tc.tile_pool(name="persist", bufs=1))
    qt_sb = persist.tile([P, NPAIR, S], F32R)   # Q^T/8, pair-major
    kt_sb = persist.tile([P, NPAIR, S], F32R)   # K^T
    v_sb = persist.tile([P, NKC, NH, DK + 1], F32R)  # V + ones col
    tri_sb = persist.tile([P, P], F32R)
    m2_sb = persist.tile([NH, NPAIR * P], F32R)

    # ---- phase 1: Q/K projections + first V window --------------------
    # V for k-chunks 0..3 is computed here (attention needs it first);
    # the remaining V windows are computed lazily inside the attention
    # loop, where the scalar-engine-bound exp leaves PE slack.
    xt_r = xt.rearrange("(o p) s -> p o s", p=P)
    with (
        tc.tile_pool(name="xpool", bufs=1) as xpool,
        tc.tile_pool(name="wpool", bufs=2) as wpool,
        tc.tile_pool(name="pjps", bufs=4, space="PSUM") as pjps,
    ):
        x_sb = xpool.tile([P, KO, S], F32R)
        # weights first: they unblock the first matmuls.  wq/wk/wv share two
        # 16KB slots (tag "w"): wv's load starts as soon as the Q pass ends.
        wq_sb = wpool.tile([P, KO, HD], F32R, tag="w")
        wq_r = wq.rearrange("(o p) m -> p o m", p=P)
        nc.scalar.dma_start(wq_sb[:, :, 0:HD // 2], wq_r[:, :, 0:HD // 2])
        nc.gpsimd.dma_start(wq_sb[:, :, HD // 2:], wq_r[:, :, HD // 2:])
        wk_sb = wpool.tile([P, KO, HD], F32R, tag="w")
        # spread the big activation load over the three DMA-capable queues
        x_engs = [nc.sync, nc.gpsimd, nc.scalar]
        for ko in range(KO):
            x_engs[ko % 3].dma_start(x_sb[:, ko, :], xt_r[:, ko, :])
        # wk and the constants are not needed until later; load behind x
        nc.sync.dma_start(wk_sb[:], wk.rearrange("(o p) m -> p o m", p=P))
        nc.gpsimd.dma_start(tri_sb[:], tri)
        nc.gpsimd.dma_start(m2_sb[:], m2)
        ones_sb = wpool.tile([P, NKC * NH], F32R, tag="ones")
        nc.gpsimd.dma_start(ones_sb[:], ones)
        nc.vector.tensor_copy(
            v_sb[:, :, :, DK],
            ones_sb.rearrange("p (a b) -> p a b", a=NKC))

        # first four Q groups run ko-outer-interleaved across the four
        # PSUM slots, so the PE advances on every x chunk as it arrives
        # instead of stalling inside one accumulation group
        first_q = [pjps.tile([P, QC], F32, tag="pj", name=f"qps{sc}")
                   for sc in range(NQC)]
        for ko in range(KO):
            for sc in range(NQC):
                nc.tensor.matmul(
                    first_q[sc][:],
                    wq_sb[:, ko, 0:P],
                    x_sb[:, ko, sc * QC:(sc + 1) * QC],
                    start=(ko == 0), stop=(ko == KO - 1),
                )
        for sc in range(NQC):
            nc.scalar.mul(
                qt_sb[:, 0, sc * QC:(sc + 1) * QC], first_q[sc][:], 1.0 / 8.0)

        for m in range(1, NPAIR):
            for sc in range(NQC):
                qps = pjps.tile([P, QC], F32, tag="pj")
                for ko in range(KO):
                    nc.tensor.matmul(
                        qps[:],
                        wq_sb[:, ko, m * P:(m + 1) * P],
                        x_sb[:, ko, sc * QC:(sc + 1) * QC],
                        start=(ko == 0), stop=(ko == KO - 1),
                    )
                # fold in the 1/sqrt(dk) softmax scale here
                # (scalar engine is idle during projections)
                nc.scalar.mul(
                    qt_sb[:, m, sc * QC:(sc + 1) * QC], qps[:], 1.0 / 8.0)

        wv_sb = wpool.tile([P, KO, HD], F32R, tag="w")
        nc.sync.dma_start(wv_sb[:], wv.rearrange("(o p) m -> p o m", p=P))

        for m in range(NPAIR):
            for sc in range(NQC):
                kps = pjps.tile([P, QC], F32, tag="pj")
                for ko in range(KO):
                    nc.tensor.matmul(
                        kps[:],
                        wk_sb[:, ko, m * P:(m + 1) * P],
                        x_sb[:, ko, sc * QC:(sc + 1) * QC],
                        start=(ko == 0), stop=(ko == KO - 1),
                    )
                nc.vector.tensor_copy(
                    kt_sb[:, m, sc * QC:(sc + 1) * QC], kps[:])

        for sc in range(4):
            vps = pjps.tile([P, HD], F32, tag="pj")
            for ko in range(KO):
                nc.tensor.matmul(
                    vps[:],
                    x_sb[:, ko, sc * P:(sc + 1) * P],
                    wv_sb[:, ko, :],
                    start=(ko == 0), stop=(ko == KO - 1),
                )
            nc.vector.tensor_copy(
                v_sb[:, sc, :, 0:DK],
                vps.rearrange("p (h e) -> p h e", h=NH))

    # ---- phase 2: attention + lazy V + normalization + out proj ------
    persist2 = ctx.enter_context(tc.tile_pool(name="persist2", bufs=1))
    ctx_sb = persist2.tile([P, NPAIR, NQC, QC], F32R)  # context^T, pair-major

    # q-chunk outer so the (PE-light, ACT-heavy) attention of chunk qc
    # overlaps the (PE-heavy) output projection of chunk qc-1 and the
    # V projection for the next k-window.
    with (
        tc.tile_pool(name="wop", bufs=1) as wop,
        tc.tile_pool(name="spps", bufs=2, space="PSUM") as spps,
        tc.tile_pool(name="otps", bufs=1, space="PSUM") as otps,
        tc.tile_pool(name="mixps", bufs=2, space="PSUM") as mixps,
        tc.tile_pool(name="ptpool", bufs=3) as ptpool,
        tc.tile_pool(name="bnpool", bufs=2) as bnpool,
        tc.tile_pool(name="osb", bufs=3) as osb,
        tc.tile_pool(name="srpool", bufs=2) as srpool,
        tc.tile_pool(name="xvpool", bufs=2) as xvpool,
        tc.tile_pool(name="wvp2", bufs=1) as wvp2,
    ):
        wo_sb = wop.tile([P, NPAIR, D], F32R)
        nc.sync.dma_start(wo_sb[:], wo.rearrange("(j p) o -> p j o", p=P))
        wv2_sb = wvp2.tile([P, KO, HD], F32R)
        nc.sync.dma_start(wv2_sb[:], wv.rearrange("(o p) m -> p o m", p=P))

        def emit_norm_oproj(qc, rcp_t, j):
            """Normalization + output projection for pair j of chunk qc."""
            rp = mixps.tile([P, QC], F32, tag="mix", name="rp")
            nc.tensor.matmul(
                rp[:], m2_sb[0:NH, j * P:(j + 1) * P],
                rcp_t[:], start=True, stop=True)
            nc.vector.tensor_mul(
                ctx_sb[:, j, qc, :], ctx_sb[:, j, qc, :], rp[:])

        def emit_oproj_group(qc, t, no):
            op = mixps.tile([P, QC], F32, tag="mix", name="op")
            for j in range(NPAIR):
                nc.tensor.matmul(
                    op[:],
                    ctx_sb[:, j, qc, t * P:(t + 1) * P],
                    wo_sb[:, j, no * QC:(no + 1) * QC],
                    start=(j == 0), stop=(j == NPAIR - 1),
                )
            o_sb = osb.tile([P, QC], F32, tag="o_sb", name="o_sb")
            nc.vector.tensor_copy(o_sb[:], op[:])
            sc = qc * (QC // P) + t
            nc.sync.dma_start(
                out[sc * P:(sc + 1) * P, no * QC:(no + 1) * QC], o_sb[:])

        pending = None  # (qc, rcp_t) whose norm+O-proj is deferred
        for qc in range(NQC):
            qcs = slice(qc * QC, (qc + 1) * QC)
            nkc = 4 * (qc + 1)
            sum_t = srpool.tile([NH, QC], F32, tag="sum")
            rcp_t = srpool.tile([NH, QC], F32R, tag="rcp")
            for j in range(NPAIR):
                # interleave the previous chunk's normalization + output
                # projection into this chunk's exp-bound attention.  All
                # four pairs must be normalized before any projection group
                # (each group contracts over every pair).
                if pending is not None:
                    pqc, prcp = pending
                    # norms start at pair 1 so pair 0's scores can fill the
                    # reciprocal wait at the chunk boundary
                    if j == 1:
                        for pj in range(NPAIR):
                            emit_norm_oproj(pqc, prcp, pj)
                    if j >= 1:
                        for no in range(D // QC):
                            emit_oproj_group(pqc, j - 1, no)
                ot0 = otps.tile([DK + 1, QC], F32, tag="ot0")
                ot1 = otps.tile([DK + 1, QC], F32, tag="ot1")
                for kc in range(nkc):
                    diag = kc >= 4 * qc
                    qlo = (kc - 4 * qc) * P if diag else 0
                    qs = slice(qc * QC + qlo, (qc + 1) * QC)
                    # both heads' scores side by side in one 2-bank psum tile
                    sp = spps.tile([P, 2 * QC], F32, tag="sp")
                    nc.tensor.matmul(
                        sp[:, qlo:QC], kt_sb[0:DK, j, kc * P:(kc + 1) * P],
                        qt_sb[0:DK, j, qs], start=True, stop=True)
                    nc.tensor.matmul(
                        sp[:, QC + qlo:], kt_sb[DK:P, j, kc * P:(kc + 1) * P],
                        qt_sb[DK:P, j, qs], start=True, stop=True)
                    pt = ptpool.tile([P, 2 * QC], F32R, tag="pt")
                    # one exp over both heads' (possibly narrowed) ranges
                    pt2 = pt.rearrange("p (a b) -> p a b", a=2)
                    sp2 = sp.rearrange("p (a b) -> p a b", a=2)
                    nc.scalar.activation(pt2[:, :, qlo:], sp2[:, :, qlo:], EXP)
                    if diag:
                        nc.vector.tensor_mul(
                            pt[:, qlo:qlo + P], pt[:, qlo:qlo + P], tri_sb[:])
                        nc.vector.tensor_mul(
                            pt[:, QC + qlo:QC + qlo + P],
                            pt[:, QC + qlo:QC + qlo + P], tri_sb[:])
                    nc.tensor.matmul(
                        ot0[:, qlo:], v_sb[:, kc, 2 * j, :], pt[:, qlo:QC],
                        start=(kc == 0), stop=(kc == nkc - 1),
                        skip_group_check=True)
                    nc.tensor.matmul(
                        ot1[:, qlo:], v_sb[:, kc, 2 * j + 1, :], pt[:, QC + qlo:],
                        start=(kc == 0), stop=(kc == nkc - 1),
                        skip_group_check=True)
                # drain: even head's context straight to its rows; the odd
                # head (and both denominator rows) bounce through SBUF and
                # DMA to their partition-shifted slots.
                bn0 = bnpool.tile([DK + 1, QC], F32R, tag="bn0")
                bn1 = bnpool.tile([DK + 1, QC], F32R, tag="bn1")
                tail = qc == NQC - 1 and j == NPAIR - 1
                if tail:
                    # final pair: denominator rows first (they gate the
                    # closing reciprocal -> normalize -> project chain)
                    nc.vector.tensor_copy(bn0[DK:DK + 1, :],
                                          ot0[DK:DK + 1, :])
                    nc.vector.tensor_copy(bn1[:], ot1[:])
                    nc.vector.tensor_copy(ctx_sb[0:DK, j, qc, :], ot0[0:DK, :])
                else:
                    nc.vector.tensor_copy(ctx_sb[0:DK, j, qc, :], ot0[0:DK, :])
                    nc.vector.tensor_copy(bn0[DK:DK + 1, :],
                                          ot0[DK:DK + 1, :])
                    nc.vector.tensor_copy(bn1[:], ot1[:])
                nc.gpsimd.dma_start(ctx_sb[DK:P, j, qc, :], bn1[0:DK, :])
                # final pair: denominators on idle queues, in parallel
                e0 = nc.scalar if tail else nc.gpsimd
                e1 = nc.sync if tail else nc.gpsimd
                e0.dma_start(sum_t[2 * j:2 * j + 1, :],
                             bn0[DK:DK + 1, :].bitcast(F32))
                e1.dma_start(sum_t[2 * j + 1:2 * j + 2, :],
                             bn1[DK:DK + 1, :].bitcast(F32))

            # the held-back projection block lands here, filling this
            # chunk's own drain waits
            if pending is not None:
                for no in range(D // QC):
                    emit_oproj_group(pending[0], NPAIR - 1, no)
            # lazy V projection for the NEXT q chunk's new k-window;
            # streams x back in from DRAM (x_sb was released after phase 1)
            if qc < NQC - 1:
                for sc in range(4 * (qc + 1), 4 * (qc + 2)):
                    xv = xvpool.tile([P, KO, P], F32R, tag="xv")
                    nc.sync.dma_start(xv[:], xt_r[:, :, sc * P:(sc + 1) * P])
                    vps = mixps.tile([P, HD], F32, tag="mix")
                    for ko in range(KO):
                        nc.tensor.matmul(
                            vps[:],
                            xv[:, ko, :],
                            wv2_sb[:, ko, :],
                            start=(ko == 0), stop=(ko == KO - 1),
                        )
                    nc.vector.tensor_copy(
                        v_sb[:, sc, :, 0:DK],
                        vps.rearrange("p (h e) -> p h e", h=NH))

            # reciprocal of this chunk's denominators; norm + O-proj are
            # deferred into the next chunk's pair loop (PE filler there)
            with nc.allow_low_precision(reason="f32r rounding of 1/denom"):
                nc.vector.reciprocal(rcp_t[:], sum_t[:])
            if qc < NQC - 1:
                pending = (qc, rcp_t)
            else:
                for j in range(NPAIR):
                    emit_norm_oproj(qc, rcp_t, j)
                for t in range(QC // P):
                    for no in range(D // QC):
                        emit_oproj_group(qc, t, no)


def build_nc():
    nc = bacc.Bacc("TRN2", target_bir_lowering=False, debug=False)
    xt = nc.dram_tensor("xt", [D, S], F32R, kind="ExternalInput").ap()
    wq = nc.dram_tensor("wq", [D, HD], F32R, kind="ExternalInput").ap()
    wk = nc.dram_tensor("wk", [D, HD], F32R, kind="ExternalInput").ap()
    wv = nc.dram_tensor("wv", [D, HD], F32R, kind="ExternalInput").ap()
    wo = nc.dram_tensor("wo", [HD, D], F32R, kind="ExternalInput").ap()
    tri = nc.dram_tensor("tri", [P, P], F32R, kind="ExternalInput").ap()
    m2 = nc.dram_tensor("m2", [NH, NPAIR * P], F32R, kind="ExternalInput").ap()
    ones = nc.dram_tensor("ones", [P, NKC * NH], F32R, kind="ExternalInput").ap()
    out = nc.dram_tensor("out", [S, D], F32, kind="ExternalOutput").ap()
    with tile.TileContext(nc) as tc:
        with ExitStack() as ctx:
            _emit(ctx, tc, xt, wq, wk, wv, wo, tri, m2, ones, out)
    nc.compile()
    return nc


def make_in_maps(x, W_q, W_k, W_v, W_o):
    x = np.asarray(x, dtype=np.float32)
    WqT = np.ascontiguousarray(np.asarray(W_q, np.float32).T)
    WkT = np.ascontiguousarray(np.asarray(W_k, np.float32).T)
    WvT = np.ascontiguousarray(np.asarray(W_v, np.float32).T)
    WoT = np.ascontiguousarray(np.asarray(W_o, np.float32).T)
    tri = np.triu(np.ones((P, P), np.float32))  # tri[k,q] = 1 where q >= k
    m2 = np.zeros((NH, NPAIR * P), np.float32)
    for j in range(NPAIR):
        m2[2 * j, j * P:j * P + DK] = 1.0
        m2[2 * j + 1, j * P + DK:(j + 1) * P] = 1.0
    in_maps = []
    for c in range(2 * B):
        b, g = c // 2, c % 2
        in_maps.append({
            "xt": np.ascontiguousarray(x[b].T),
            "wq": np.ascontiguousarray(WqT[:, g * HD:(g + 1) * HD]),
            "wk": np.ascontiguousarray(WkT[:, g * HD:(g + 1) * HD]),
            "wv": np.ascontiguousarray(WvT[:, g * HD:(g + 1) * HD]),
            "wo": np.ascontiguousarray(WoT[g * HD:(g + 1) * HD, :]),
            "tri": tri,
            "m2": m2,
            "ones": np.ones((P, NKC * NH), np.float32),
        })
    return in_maps


def get_runner():
    """Build (once) and cache a jitted 8-core executor for the bass program.

    Returns run(in_maps) -> list of per-core {name: np.ndarray} outputs.
    Mirrors concourse.bass2jax.run_bass_via_pjrt but caches the jitted
    callable so repeat kernel() calls skip re-lowering/compiling.
    """
    if "runner" in _CACHE:
        return _CACHE["runner"]
    import jax
    from jax.experimental.shard_map import shard_map
    from jax.sharding import Mesh, PartitionSpec
    from concourse import mybir as _mb
    from concourse.bass2jax import (
        _bass_exec_p, install_neuronx_cc_hook, partition_id_tensor)

    install_neuronx_cc_hook()
    nc = build_nc()
    n_cores = 2 * B

    partition_name = (nc.partition_id_tensor.name
                      if nc.partition_id_tensor else None)
    in_names, out_names, out_avals = [], [], []
    for alloc in nc.m.functions[0].allocations:
        if not isinstance(alloc, _mb.MemoryLocationSet):
            continue
        name = alloc.memorylocations[0].name
        if alloc.kind == "ExternalInput":
            if name != partition_name:
                in_names.append(name)
        elif alloc.kind == "ExternalOutput":
            out_names.append(name)
            out_avals.append(jax.core.ShapedArray(
                tuple(alloc.tensor_shape), _mb.dt.np(alloc.dtype)))
    n_params = len(in_names)
    all_names = in_names + out_names
    if partition_name is not None:
        all_names = all_names + [partition_name]

    def _body(*args):
        operands = list(args)
        if partition_name is not None:
            operands.append(partition_id_tensor())
        outs = _bass_exec_p.bind(
            *operands,
            out_avals=tuple(out_avals),
            in_names=tuple(all_names),
            out_names=tuple(out_names),
            lowering_input_output_aliases=(),
            sim_require_finite=True,
            sim_require_nnan=True,
            nc=nc,
        )
        return tuple(outs)

    devices = jax.devices()[:n_cores]
    mesh = Mesh(np.asarray(devices), ("core",))
    n_outs = len(out_names)
    sharded = jax.jit(
        shard_map(
            _body, mesh=mesh,
            in_specs=(PartitionSpec("core"),) * (n_params + n_outs),
            out_specs=(PartitionSpec("core"),) * n_outs,
            check_rep=False,
        ),
        donate_argnums=tuple(range(n_params, n_params + n_outs)),
        keep_unused=True,
    )

    def run(in_maps, device_arrays=None):
        concat_in = device_arrays if device_arrays is not None else [
            np.concatenate([np.asarray(in_maps[c][i_name])
                            for c in range(n_cores)], axis=0)
            for i_name in in_names
        ]
        concat_zeros = [
            np.zeros((n_cores * av.shape[0], *av.shape[1:]), av.dtype)
            for av in out_avals
        ]
        out_arrs = sharded(*concat_in, *concat_zeros)
        return [
            {name: np.asarray(out_arrs[i]).reshape(
                n_cores, *out_avals[i].shape)[c]
             for i, name in enumerate(out_names)}
            for c in range(n_cores)
        ]

    _CACHE["runner"] = (run, in_names, out_avals)
    return _CACHE["runner"]


def _run_cores(in_maps):
    """Execute the 8-core program; returns per-core {name: np.ndarray}."""
    from concourse._compat import axon_active
    if axon_active():
        # remote-accelerator proxy: use the cached jitted PJRT executor so
        # repeat calls skip re-lowering/compiling
        run, _, _ = get_runner()
        return run(in_maps)
    # native path (local /dev/neuron*): run_bass_kernel_spmd handles NEFF
    # compile caching + device execution directly
    if "nc" not in _CACHE:
        _CACHE["nc"] = build_nc()
    res = run_bass_kernel_spmd(_CACHE["nc"], in_maps, core_ids=list(range(2 * B)))
    _CACHE["last_exec_time_ns"] = res.exec_time_ns
    return res.results


def kernel(x, W_q, W_k, W_v, W_o):
    in_maps = make_in_maps(x, W_q, W_k, W_v, W_o)
    results = _run_cores(in_maps)
    out = np.empty((B, S, D), np.float32)
    for b in range(B):
        out[b] = results[2 * b]["out"] + results[2 * b + 1]["out"]
    return out

